# revision 1
# baseline (speedup 1.0000x reference)
"""Autoregressive GRU decoder on 8 TRN2 NeuronCores (data-parallel over batch).

Math (per step, reference semantics):
    px   = x * w_proj + b_proj                      # x is scalar per row
    gi   = px @ W_ih.T + b_ih                       # rank-1 in x:
         = x * u + c,   u = W_ih @ w_proj,  c = W_ih @ b_proj + b_ih
    gh   = h @ W_hh.T + b_hh
    r    = sigmoid(gi_r + gh_r);  z = sigmoid(gi_z + gh_z)
    n    = tanh(gi_n + r * gh_n')                   # gh_n' = gh_n (incl b_hh_n)
    h    = n + z * (h - n)
    pred = h @ w_out + b_out;  x_next = pred

Device layout is feature-major (hidden dim on partitions, batch on the free
dim) so the recurrent matmul needs no transposes; the host pre-transposes
encoder_out and W_hh and precomputes u/c (tiny: ~0.4 MFLOP of weight prep).

Implementation notes (per core: 2048 batch rows, 4 column-chunks of 512):
- bf16 datapath, f32 PSUM accumulation (rel err vs f32 reference ~7.5e-3).
- PE: per chunk 8x K=128 main matmuls into r/z PSUM, 4x into n PSUM, plus the
  rank-1 [u*x; d*1] folds as 4 K=2 matmuls packed into distinct 32-row groups
  via tile_position (concurrent on silicon; x is mirrored to partitions
  32/64/96 by one strided SBUF DMA from xb), and a K=256 matvec for pred.
- ACT: fused [128,1024] sigmoids (r, z) and tanh; per-partition bias APs carry
  c_n; scalar_tensor_tensor on DVE carries b_hh_n and the gi_n add.
- Pool(GpSimd): partition_broadcast of the step input x, and the h-update
  chain (h-n, z*(h-n), h_new) to offload DVE.
- PSUM: 8 banks = g_r(2) + g_z(2) + g_n(2 dbl-buffered) + pred(2 slots);
  r/z split into separate tensors so the next chunk's matmuls only wait on
  the sigmoid that drained first.
- Emission is software-pipelined: each chunk's pred/x-write is emitted one
  chunk late so the PE stream never waits on the gate chain (sim: zero PE
  gaps >200ns in steady state; 95% PE occupancy at serial pricing).
"""

import sys

import numpy as np

if "/opt/trn_rl_repo" not in sys.path:
    sys.path.insert(0, "/opt/trn_rl_repo")

N = 16384
H = 256
H3 = 3 * H
T = 24
NCORES = 8
R = N // NCORES  # 2048 rows per core
CH = 512  # batch-column chunk (one PSUM bank at f32)
NCH = R // CH

_CACHE: dict = {}


def _build():
    import concourse.bass as bass
    import concourse.bacc as bacc
    import concourse.mybir as mybir
    from concourse.tile import TileContext

    f32 = mybir.dt.float32
    bf16 = mybir.dt.bfloat16
    AF = mybir.ActivationFunctionType
    OP = mybir.AluOpType

    nc = bacc.Bacc()

    h0T_d = nc.declare_dram_parameter("h0T", [H, R], bf16, isOutput=False)
    whhT_d = nc.declare_dram_parameter("whhT", [H, H3], bf16, isOutput=False)
    s3x2_d = nc.declare_dram_parameter("s3x2", [128, H3], bf16, isOutput=False)
    uc_d = nc.declare_dram_parameter("uc", [2, 128, 3], f32, isOutput=False)
    wo_d = nc.declare_dram_parameter("wo", [2, 128, 1], bf16, isOutput=False)
    bo_d = nc.declare_dram_parameter("bo", [1, 1], f32, isOutput=False)
    out_d = nc.declare_dram_parameter("out", [T, R], bf16, isOutput=True)

    from contextlib import ExitStack

    with TileContext(nc) as tc, ExitStack() as stack:
        persist = stack.enter_context(tc.tile_pool(name="persist", bufs=1))

        def mk(shape, name, dt=None):
            return persist.tile(shape, dt if dt is not None else bf16,
                                name=name, tag=name)

        # ---- persistent SBUF state ----
        hT0 = mk([128, R], "hT0")   # hidden rows 0:128, batch on free dim
        hT1 = mk([128, R], "hT1")   # hidden rows 128:256
        wT0 = mk([128, H3], "wT0")  # W_hh.T rows 0:128
        wT1 = mk([128, H3], "wT1")  # W_hh.T rows 128:256
        s3x2 = mk([128, H3], "s3x2")  # [u;d] row pairs at partitions {32m,32m+1}
        uc0 = mk([128, 3], "uc0", f32)  # cols: u_n, c_n, b_hh_n (hidden 0:128)
        uc1 = mk([128, 3], "uc1", f32)
        wo0 = mk([128, 1], "wo0")
        wo1 = mk([128, 1], "wo1")
        bo = mk([1, 1], "bo", f32)
        xob = mk([128, R], "xob")   # rows 32m = x (DMA), rows 32m+1 = 1.0
        # double-buffered x rows: row0 = current x (pred), row1 = ones
        xoA = mk([2, R], "xoA")
        xoB = mk([2, R], "xoB")
        xb = mk([128, R], "xb")     # x broadcast across partitions
        gi0 = mk([128, R], "gi0")   # x*u_n + c_n (hidden 0:128)
        gi1 = mk([128, R], "gi1")

        nc.sync.dma_start(out=hT0[:], in_=h0T_d[0:128, :])
        nc.sync.dma_start(out=hT1[:], in_=h0T_d[128:256, :])
        nc.sync.dma_start(out=wT0[:], in_=whhT_d[0:128, :])
        nc.sync.dma_start(out=wT1[:], in_=whhT_d[128:256, :])
        nc.sync.dma_start(out=s3x2[:], in_=s3x2_d[:])
        nc.sync.dma_start(out=uc0[:], in_=uc_d[0])
        nc.sync.dma_start(out=uc1[:], in_=uc_d[1])
        nc.sync.dma_start(out=wo0[:], in_=wo_d[0])
        nc.sync.dma_start(out=wo1[:], in_=wo_d[1])
        nc.sync.dma_start(out=bo[:], in_=bo_d[:])
        nc.vector.memset(xob[:], 1.0)  # x rows overwritten per chunk
        nc.vector.memset(xoA[0:2, :], 1.0)   # row1 stays 1.0 forever
        nc.vector.memset(xoA[0:1, :], 0.0)   # row0 = x_0 = 0
        nc.vector.memset(xoB[0:2, :], 1.0)

        with (
            tc.tile_pool(name="gr", bufs=1, space="PSUM") as grpool,
            tc.tile_pool(name="gz", bufs=1, space="PSUM") as gzpool,
            tc.tile_pool(name="gn", bufs=2, space="PSUM") as gnpool,
            tc.tile_pool(name="sm", bufs=2, space="PSUM") as smpool,
            tc.tile_pool(name="rz", bufs=3) as rzpool,
            tc.tile_pool(name="wk", bufs=6) as wkpool,
        ):
            def emit_tail(pend):
                """pred + x/out write for a finished chunk (pipelined)."""
                tt, cc, xo_t = pend
                slc = slice(cc * CH, (cc + 1) * CH)
                pp = smpool.tile([1, CH], f32, tag="sm", name="pp")
                nc.tensor.matmul(pp[:], wo0[:], hT0[:, slc],
                                 start=True, stop=False)
                nc.tensor.matmul(pp[:], wo1[:], hT1[:, slc],
                                 start=False, stop=True)
                nc.vector.tensor_scalar_add(xo_t[0:1, slc], pp[:], bo[0:1, 0:1])
                if cc == NCH - 1:
                    nc.sync.dma_start(out=out_d[tt : tt + 1, :], in_=xo_t[0:1, :])

            pending = None
            for t in range(T):
                xin = xoA if t % 2 == 0 else xoB   # x_t lives here
                xout = xoB if t % 2 == 0 else xoA  # pred_t written here

                def prefetch(cc, xin=xin):
                    """x broadcast + gi_n for chunk cc; also place x at
                    partitions 32/64/96 for the packed rank-1 matmuls."""
                    slc = slice(cc * CH, (cc + 1) * CH)
                    nc.gpsimd.partition_broadcast(xb[:, slc], xin[0:1, slc])
                    nc.sync.dma_start(out=xob[32:128:32, slc],
                                      in_=xb[32:128:32, slc])
                    nc.vector.tensor_scalar(gi0[:, slc], xb[:, slc],
                                            uc0[:, 0:1], uc0[:, 1:2],
                                            OP.mult, OP.add)
                    nc.vector.tensor_scalar(gi1[:, slc], xb[:, slc],
                                            uc1[:, 0:1], uc1[:, 1:2],
                                            OP.mult, OP.add)

                # chunks 0-2 read x slices whose tsadd was emitted by the end
                # of step t-1; chunk 3's lands after the pending flush below
                for cc in range(NCH - 1):
                    prefetch(cc)
                for c in range(NCH):
                    sl = slice(c * CH, (c + 1) * CH)
                    gr = grpool.tile([128, 2 * CH], f32, tag="gr", name="gr")
                    gz = gzpool.tile([128, 2 * CH], f32, tag="gz", name="gz")
                    rz = rzpool.tile([128, 4 * CH], bf16, tag="rz", name="rz")
                    def gsl(m):
                        return (gr if m < 2 else gz)[
                            :, (m % 2) * CH : (m % 2 + 1) * CH]

                    for m in range(4):
                        ms = slice(m * 128, (m + 1) * 128)
                        nc.tensor.matmul(gsl(m), wT0[:, ms], hT0[:, sl],
                                         start=True, stop=False)
                        nc.tensor.matmul(gsl(m), wT1[:, ms], hT1[:, sl],
                                         start=False, stop=False)
                    # rank-1 [x;1] fold: K=2 matmuls packed into distinct
                    # 32-row groups of the PE array -> run concurrently
                    for m in range(4):
                        mov = (xin[0:2, sl] if m == 0
                               else xob[32 * m : 32 * m + 2, sl])
                        nc.tensor.matmul(gsl(m),
                                         s3x2[32 * m : 32 * m + 2,
                                              m * 128 : (m + 1) * 128],
                                         mov,
                                         start=False, stop=True,
                                         tile_position=(32 * m, 0))
                    nc.scalar.activation(rz[:, 0 : 2 * CH], gr[:], AF.Sigmoid)
                    nc.scalar.activation(rz[:, 2 * CH : 4 * CH],
                                         gz[:], AF.Sigmoid)
                    gns = []
                    for tl in range(2):
                        gn = gnpool.tile([128, CH], f32, tag="gn", name="gn")
                        ms = slice((4 + tl) * 128, (5 + tl) * 128)
                        nc.tensor.matmul(gn[:], wT0[:, ms], hT0[:, sl],
                                         start=True, stop=False)
                        nc.tensor.matmul(gn[:], wT1[:, ms], hT1[:, sl],
                                         start=False, stop=True)
                        gns.append(gn)
                    # previous chunk's pred now that PE has fresh work queued
                    if pending is not None:
                        emit_tail(pending)
                    if c == 0:
                        prefetch(NCH - 1)
                    ta2 = wkpool.tile([128, 2 * CH], bf16, tag="ta2", name="ta2")
                    nn2 = wkpool.tile([128, 2 * CH], bf16, tag="nn2", name="nn2")
                    for tl, (hT, uc, gi) in enumerate(
                            ((hT0, uc0, gi0), (hT1, uc1, gi1))):
                        gn = gns[tl]
                        rr = rz[:, tl * CH : (tl + 1) * CH]
                        # rh = (gh_n + b_hh_n) * r
                        rh = wkpool.tile([128, CH], bf16, tag="rh", name="rh")
                        nc.vector.scalar_tensor_tensor(
                            rh[:], gn[:], uc[:, 2:3], rr, OP.add, OP.mult)
                        # ta = gi_n + rh
                        nc.vector.tensor_tensor(
                            ta2[:, tl * CH : (tl + 1) * CH], gi[:, sl], rh[:],
                            OP.add)
                    # n = tanh(ta) for both hidden tiles in one LUT pass
                    nc.scalar.activation(nn2[:], ta2[:], AF.Tanh)
                    for tl, hT in enumerate((hT0, hT1)):
                        nn = nn2[:, tl * CH : (tl + 1) * CH]
                        zz = rz[:, (2 + tl) * CH : (3 + tl) * CH]
                        hmn = wkpool.tile([128, CH], bf16, tag="hmn", name="hmn")
                        nc.gpsimd.tensor_tensor(hmn[:], hT[:, sl], nn, OP.subtract)
                        zh = wkpool.tile([128, CH], bf16, tag="zh", name="zh")
                        nc.gpsimd.tensor_tensor(zh[:], zz, hmn[:], OP.mult)
                        nc.gpsimd.tensor_tensor(hT[:, sl], nn, zh[:], OP.add)
                    pending = (t, c, xout)
            emit_tail(pending)

    nc.compile()
    return nc


def _prep_maps(encoder_out, w_proj, b_proj, W_ih, b_ih, W_hh, b_hh, w_out, b_out):
    f = np.float32
    u = (W_ih @ w_proj).astype(f)                    # [768]
    cvec = (W_ih @ b_proj + b_ih).astype(f)          # [768]
    s3 = np.zeros((2, H3), f)
    s3[0, : 2 * H] = u[: 2 * H]
    s3[1, : 2 * H] = cvec[: 2 * H] + b_hh[: 2 * H]
    s3[1, 2 * H :] = b_hh[2 * H :]
    s3x2 = np.zeros((128, H3), f)
    for m in range(4):
        s3x2[32 * m : 32 * m + 2, :] = s3
    uc = np.zeros((2, 128, 3), f)
    uc[0, :, 0] = u[2 * H : 2 * H + 128]
    uc[0, :, 1] = cvec[2 * H : 2 * H + 128]
    uc[0, :, 2] = b_hh[2 * H : 2 * H + 128]
    uc[1, :, 0] = u[2 * H + 128 :]
    uc[1, :, 1] = cvec[2 * H + 128 :]
    uc[1, :, 2] = b_hh[2 * H + 128 :]
    import ml_dtypes

    bf = ml_dtypes.bfloat16
    wo = np.ascontiguousarray(w_out.astype(f).reshape(2, 128, 1)).astype(bf)
    bo = b_out.astype(f).reshape(1, 1)
    whhT = np.ascontiguousarray(W_hh.astype(f).T).astype(bf)  # [256, 768]
    s3x2 = s3x2.astype(bf)
    maps = []
    for i in range(NCORES):
        h0T = np.ascontiguousarray(
            encoder_out[i * R : (i + 1) * R].astype(f).T
        ).astype(bf)
        maps.append(
            dict(h0T=h0T, whhT=whhT, s3x2=s3x2, uc=uc, wo=wo, bo=bo)
        )
    return maps


def _run(inputs, trace=False, **kw):
    import time

    from concourse.bass_utils import run_bass_kernel_spmd

    if "nc" not in _CACHE:
        _CACHE["nc"] = _build()
    nc = _CACHE["nc"]
    in_maps = _prep_maps(**inputs)
    res = None
    for attempt, pause in enumerate((0, 30, 120)):
        if pause:
            time.sleep(pause)  # transient NRT/axon device errors self-recover
        try:
            res = run_bass_kernel_spmd(nc, in_maps, core_ids=list(range(NCORES)),
                                       trace=trace, **kw)
            break
        except Exception:
            if attempt == 2:
                raise
    full = np.empty((N, T), np.float32)
    for i in range(NCORES):
        o = np.asarray(res.results[i]["out"]).astype(np.float32)
        full[i * R : (i + 1) * R] = o.T
    return full, res


def kernel(**inputs):
    inputs = {k: np.asarray(v) for k, v in inputs.items()}
    full, _ = _run(inputs)
    return full



# revision 4
# speedup vs baseline: 1.3242x; 1.3242x over previous
"""Autoregressive GRU decoder on 8 TRN2 NeuronCores (data-parallel over batch).

Math (per step, reference semantics):
    gi   = x*u + c  (rank-1: u = W_ih@w_proj, c = W_ih@b_proj + b_ih)
    gh   = h @ W_hh.T + b_hh
    r    = sigmoid(gi_r + gh_r);  z = sigmoid(gi_z + gh_z)
    n    = tanh(gi_n + r * gh_n)
    h    = n + z*(h - n);  pred = h @ w_out + b_out;  x_next = pred

Device-side restructurings:
- x_t = w_out.h_t + b_out exactly (t>=1), so the rank-1 input term for r/z
  folds INTO the recurrent weights: W'_rz = W_hh_rz^T + w_out u_rz^T (b_out
  absorbed into biases).  No per-step rank-1 matmuls.
- Gate matmuls run fp8-e4m3 DoubleRow (K=256/instr, 0.5 cyc/row).  Weights
  are split W_hi + W_lo (same scale) to cancel weight-quantization error; the
  moving operand is a shadow h8 = Q(8h) refreshed per step.  The bf16 state
  stays the elementwise source of truth (a pure-fp8 state diverges, 6.5e-2).
- Gate biases enter PSUM via K=1 ones-matmuls; the fp8 descale rides the
  ACT scale field and DVE per-partition scalars.
- x is produced broadcast over partitions by a matmul whose stationary is
  w_out replicated across output rows, streaming bf16 nn / zh (linearity:
  w_out.h' = w_out.nn + w_out.zh) so pred quality never touches fp8.
- preds for the OUTPUT are computed on the host from the DMA'd nn/zh
  tensors (f32 matvec per step); no PSUM row extraction on device.
- PSUM: gr/gz rotate in a [128,1024] f32 bufs=2 pool, gn [128,1024] bufs=1,
  xb [128,512] bufs=2 -> exactly 8 banks.
- Per-chunk engine budget: PE ~2985ns (16 MMs), ACT ~3114 (2 sigmoid+tanh),
  DVE ~2950 (rh stt, affine_then_add, h8 half), Pool ~2990 (h-update, h8
  half). nn/zh out-DMAs issue from the Pool queue (25ns) onto DMA engines.
"""

import sys

import numpy as np

if "/opt/trn_rl_repo" not in sys.path:
    sys.path.insert(0, "/opt/trn_rl_repo")

N = 16384
H = 256
T = 24
NCORES = 8
R = N // NCORES  # 2048 rows per core
CH = 512
NCH = R // CH

SW = 16.0            # fp8 weight scale (hi and lo at the same scale)
SH = 8.0             # fp8 h-shadow scale
DESC = 1.0 / (SW * SH)
IDESC = SW * SH

_CACHE: dict = {}


def _build():
    import concourse.bacc as bacc
    import concourse.mybir as mybir
    from concourse.tile import TileContext

    f32 = mybir.dt.float32
    bf16 = mybir.dt.bfloat16
    f8 = mybir.dt.float8e4
    AF = mybir.ActivationFunctionType
    OP = mybir.AluOpType
    DR = mybir.MatmulPerfMode.DoubleRow

    nc = bacc.Bacc()

    h0T_d = nc.declare_dram_parameter("h0T", [128, 2, R], bf16, isOutput=False)
    h08_d = nc.declare_dram_parameter("h08", [128, 2, R], f8, isOutput=False)
    wrz_d = nc.declare_dram_parameter("wrz", [2, 128, 2, 512], f8, isOutput=False)
    wrz0_d = nc.declare_dram_parameter("wrz0", [2, 128, 2, 512], f8, isOutput=False)
    wn_d = nc.declare_dram_parameter("wn", [2, 128, 2, 256], f8, isOutput=False)
    bias_d = nc.declare_dram_parameter("bias", [1, 8, 128], bf16, isOutput=False)
    worep_d = nc.declare_dram_parameter("worep", [128, 256], bf16, isOutput=False)
    # cols: 0,1 u_n/d | 2,3 c_nf/d | 4,5 c_n0/d | 6,7 b_n/d   (per half)
    scal_d = nc.declare_dram_parameter("scal", [128, 8], f32, isOutput=False)
    nn_d = nc.declare_dram_parameter("nn", [T, 128, 2, R], bf16, isOutput=True)
    zh_d = nc.declare_dram_parameter("zh", [T, 128, 2, R], bf16, isOutput=True)

    from contextlib import ExitStack

    with TileContext(nc) as tc, ExitStack() as stack:
        persist = stack.enter_context(tc.tile_pool(name="persist", bufs=1))

        def mk(shape, name, dt=bf16):
            return persist.tile(shape, dt, name=name, tag=name)

        hT = mk([128, 2, R], "hT")
        h8 = mk([128, 2, R], "h8", f8)
        nnP = [mk([128, 2, R], "nnA"), mk([128, 2, R], "nnB")]
        zhP = [mk([128, 2, R], "zhA"), mk([128, 2, R], "zhB")]
        wrz = mk([128, 2, 2, 512], "wrz", f8)
        wrz0 = mk([128, 2, 2, 512], "wrz0", f8)
        wn = mk([128, 2, 2, 256], "wn", f8)
        biasr = mk([1, 8, 128], "biasr")
        worep = mk([128, 256], "worep")
        scal = mk([128, 8], "scal", f32)
        ones = mk([1, CH], "ones")
        zeros = mk([128, CH], "zeros")

        nc.sync.dma_start(out=hT[:], in_=h0T_d[:])
        nc.sync.dma_start(out=h8[:], in_=h08_d[:])
        nc.sync.dma_start(out=wrz[:, 0], in_=wrz_d[0])
        nc.sync.dma_start(out=wrz[:, 1], in_=wrz_d[1])
        nc.sync.dma_start(out=wrz0[:, 0], in_=wrz0_d[0])
        nc.sync.dma_start(out=wrz0[:, 1], in_=wrz0_d[1])
        nc.sync.dma_start(out=wn[:, 0], in_=wn_d[0])
        nc.sync.dma_start(out=wn[:, 1], in_=wn_d[1])
        nc.sync.dma_start(out=biasr[:], in_=bias_d[:])
        nc.sync.dma_start(out=worep[:], in_=worep_d[:])
        nc.sync.dma_start(out=scal[:], in_=scal_d[:])
        nc.vector.memset(ones[:], 1.0)
        nc.vector.memset(zeros[:], 0.0)

        with (
            tc.tile_pool(name="grz", bufs=2, space="PSUM") as grzpool,
            tc.tile_pool(name="gn", bufs=1, space="PSUM") as gnpool,
            tc.tile_pool(name="xb", bufs=2, space="PSUM") as xbpool,
            tc.tile_pool(name="rz", bufs=3) as rzpool,
            tc.tile_pool(name="wk", bufs=8) as wkpool,
        ):
            pending = None

            def flush(p):
                """Gate chain tail for a chunk (runs one chunk late)."""
                t, c, gnt, xbt, rz2 = p
                nn = nnP[t % 2]
                zh = zhP[t % 2]
                sl = slice(c * CH, (c + 1) * CH)
                ta2 = wkpool.tile([128, 2, CH], bf16, tag="ta2", name="ta2")
                for hh in range(2):
                    rr = rz2[:, hh * CH : (hh + 1) * CH]
                    rh = wkpool.tile([128, CH], bf16, tag="rh", name="rh")
                    nc.vector.scalar_tensor_tensor(
                        rh[:], gnt[:, hh * CH : (hh + 1) * CH],
                        scal[:, 6 + hh : 7 + hh], rr, OP.add, OP.mult)
                    xin = xbt[:] if t > 0 else zeros[:]
                    bcol = (2 if t > 0 else 4) + hh
                    nc.vector.affine_then_add(
                        ta2[:, hh], xin, rh[:],
                        scal[:, hh : hh + 1], scal[:, bcol : bcol + 1])
                nc.scalar.activation(nn[:, :, sl], ta2[:], AF.Tanh, scale=DESC)
                for hh in range(2):
                    zz = rz2[:, (2 + hh) * CH : (3 + hh) * CH]
                    nslc = nn[:, hh, sl]
                    hslc = hT[:, hh, sl]
                    hmn = wkpool.tile([128, CH], bf16, tag="hmn", name="hmn")
                    nc.gpsimd.tensor_tensor(hmn[:], hslc, nslc, OP.subtract)
                    nc.gpsimd.tensor_tensor(zh[:, hh, sl], zz, hmn[:], OP.mult)
                    nc.gpsimd.tensor_tensor(hslc, nslc, zh[:, hh, sl], OP.add)
                nc.vector.tensor_scalar(h8[:, 0, sl], hT[:, 0, sl], SH, 0.0,
                                        OP.mult, OP.add)
                nc.gpsimd.tensor_scalar(h8[:, 1, sl], hT[:, 1, sl], SH, 0.0,
                                        OP.mult, OP.add)
                if c == NCH - 1:
                    nc.sync.dma_start(out=nn_d[t], in_=nn[:])
                    nc.sync.dma_start(out=zh_d[t], in_=zh[:])

            for t in range(T):
                w = wrz0 if t == 0 else wrz
                bbase = 4 if t == 0 else 0
                nn = nnP[(t + 1) % 2]
                zh = zhP[(t + 1) % 2]
                for c in range(NCH):
                    sl = slice(c * CH, (c + 1) * CH)
                    xbt = None
                    if t > 0:
                        xbt = xbpool.tile([128, CH], f32, tag="xb", name="xb")
                        nc.tensor.matmul(xbt[:], worep[:, 0:128], nn[:, 0, sl],
                                         start=True, stop=False)
                        nc.tensor.matmul(xbt[:], worep[:, 128:256], nn[:, 1, sl],
                                         start=False, stop=False)
                        nc.tensor.matmul(xbt[:], worep[:, 0:128], zh[:, 0, sl],
                                         start=False, stop=False)
                        nc.tensor.matmul(xbt[:], worep[:, 128:256], zh[:, 1, sl],
                                         start=False, stop=True)
                    gr = grzpool.tile([128, 2 * CH], f32, tag="grz", name="gr")
                    gz = grzpool.tile([128, 2 * CH], f32, tag="grz", name="gz")
                    gnt = gnpool.tile([128, 2 * CH], f32, tag="gn", name="gn")
                    for j in range(4):
                        g = gr if j < 2 else gz
                        out = g[:, (j % 2) * CH : (j % 2 + 1) * CH]
                        nc.tensor.matmul(out, biasr[0:1, bbase + j, :], ones[:],
                                         start=True, stop=False)
                        nc.tensor.matmul(out, w[:, 0, :, j * 128 : (j + 1) * 128],
                                         h8[:, :, sl], start=False, stop=False,
                                         perf_mode=DR)
                        nc.tensor.matmul(out, w[:, 1, :, j * 128 : (j + 1) * 128],
                                         h8[:, :, sl], start=False, stop=True,
                                         perf_mode=DR)
                    for hh in range(2):
                        out = gnt[:, hh * CH : (hh + 1) * CH]
                        nc.tensor.matmul(out, wn[:, 0, :, hh * 128 : (hh + 1) * 128],
                                         h8[:, :, sl], start=True, stop=False,
                                         perf_mode=DR)
                        nc.tensor.matmul(out, wn[:, 1, :, hh * 128 : (hh + 1) * 128],
                                         h8[:, :, sl], start=False, stop=True,
                                         perf_mode=DR)
                    rz2 = rzpool.tile([128, 4 * CH], bf16, tag="rz2", name="rz2")
                    nc.scalar.activation(rz2[:, 0 : 2 * CH], gr[:], AF.Sigmoid,
                                         scale=DESC)
                    nc.scalar.activation(rz2[:, 2 * CH : 4 * CH], gz[:],
                                         AF.Sigmoid, scale=DESC)
                    if pending is not None:
                        flush(pending)
                    pending = (t, c, gnt, xbt, rz2)
            flush(pending)

    nc.compile()
    return nc


def _q8(x, scale):
    import ml_dtypes

    return (np.asarray(x, np.float32) * scale).astype(ml_dtypes.float8_e4m3)


def _prep_maps(encoder_out, w_proj, b_proj, W_ih, b_ih, W_hh, b_hh, w_out, b_out):
    import ml_dtypes

    f = np.float32
    bf = ml_dtypes.bfloat16
    W_hh = np.asarray(W_hh, f)
    w_out_f = np.asarray(w_out, f)
    u = (np.asarray(W_ih, f) @ np.asarray(w_proj, f)).astype(f)
    cvec = (np.asarray(W_ih, f) @ np.asarray(b_proj, f) + np.asarray(b_ih, f)).astype(f)
    b_hh = np.asarray(b_hh, f)
    bo = float(np.asarray(b_out, f)[0])

    WT = np.ascontiguousarray(W_hh.T)                # [256, 768]
    Wrz_fold = WT[:, : 2 * H] + np.outer(w_out_f, u[: 2 * H])
    Wrz_0 = WT[:, : 2 * H]
    Wn = WT[:, 2 * H :]
    brz_fold = b_hh[: 2 * H] + cvec[: 2 * H] + u[: 2 * H] * bo
    brz_0 = b_hh[: 2 * H] + cvec[: 2 * H]
    u_n = u[2 * H :]
    c_nf = cvec[2 * H :] + u_n * bo
    c_n0 = cvec[2 * H :]
    b_n = b_hh[2 * H :]

    def pack_w(W, m):
        hi = _q8(W, SW)
        lo = _q8(W - hi.astype(f) / SW, SW)
        out = np.zeros((2, 128, 2, m), ml_dtypes.float8_e4m3)
        for i, wq in enumerate((hi, lo)):
            out[i, :, 0, :] = wq[0:128, :]
            out[i, :, 1, :] = wq[128:256, :]
        return out

    wrz = pack_w(Wrz_fold, 512)
    wrz0 = pack_w(Wrz_0, 512)
    wn = pack_w(Wn, 256)

    bias = np.zeros((1, 8, 128), f)
    for j in range(4):
        bias[0, j] = brz_fold[j * 128 : (j + 1) * 128] * IDESC
        bias[0, 4 + j] = brz_0[j * 128 : (j + 1) * 128] * IDESC
    bias = bias.astype(bf)

    worep = np.zeros((128, 256), f)
    worep[:, 0:128] = np.tile(w_out_f[0:128][:, None], (1, 128))
    worep[:, 128:256] = np.tile(w_out_f[128:256][:, None], (1, 128))
    worep = worep.astype(bf)

    scal = np.zeros((128, 8), f)
    scal[:, 0] = u_n[0:128] * IDESC
    scal[:, 1] = u_n[128:256] * IDESC
    scal[:, 2] = c_nf[0:128] * IDESC
    scal[:, 3] = c_nf[128:256] * IDESC
    scal[:, 4] = c_n0[0:128] * IDESC
    scal[:, 5] = c_n0[128:256] * IDESC
    scal[:, 6] = b_n[0:128] * IDESC
    scal[:, 7] = b_n[128:256] * IDESC

    enc = np.asarray(encoder_out, f)
    maps = []
    for i in range(NCORES):
        blk = enc[i * R : (i + 1) * R].T  # [256, R]
        h0T = np.zeros((128, 2, R), f)
        h0T[:, 0, :] = blk[0:128]
        h0T[:, 1, :] = blk[128:256]
        maps.append(dict(
            h0T=h0T.astype(bf),
            h08=(h0T * SH).astype(ml_dtypes.float8_e4m3),
            wrz=wrz, wrz0=wrz0, wn=wn, bias=bias, worep=worep, scal=scal,
        ))
    return maps, (w_out_f.astype(bf).astype(f), bo)


def _run(inputs, trace=False, **kw):
    import time

    from concourse.bass_utils import run_bass_kernel_spmd

    if "nc" not in _CACHE:
        _CACHE["nc"] = _build()
    nc = _CACHE["nc"]
    in_maps, (wo_h, bo) = _prep_maps(**inputs)
    res = None
    for attempt, pause in enumerate((0, 30, 120)):
        if pause:
            time.sleep(pause)  # transient NRT/axon device errors self-recover
        try:
            res = run_bass_kernel_spmd(nc, in_maps, core_ids=list(range(NCORES)),
                                       trace=trace, **kw)
            break
        except Exception:
            if attempt == 2:
                raise
    full = np.empty((N, T), np.float32)
    for i in range(NCORES):
        nn_o = np.asarray(res.results[i]["nn"]).astype(np.float32)  # [T,128,2,R]
        zh_o = np.asarray(res.results[i]["zh"]).astype(np.float32)
        hsum = nn_o + zh_o                                           # h_{t+1}
        h_flat = hsum.transpose(0, 3, 2, 1).reshape(T, R, 2 * 128)
        preds = h_flat @ wo_h + bo                                   # [T, R]
        full[i * R : (i + 1) * R] = preds.T
    return full, res


def kernel(**inputs):
    inputs = {k: np.asarray(v) for k, v in inputs.items()}
    full, _ = _run(inputs)
    return full


# revision 10
# speedup vs baseline: 1.3645x; 1.0304x over previous
"""Autoregressive GRU decoder on 8 TRN2 NeuronCores (data-parallel over batch).

Math (per step, reference semantics):
    gi   = x*u + c  (rank-1: u = W_ih@w_proj, c = W_ih@b_proj + b_ih)
    gh   = h @ W_hh.T + b_hh
    r    = sigmoid(gi_r + gh_r);  z = sigmoid(gi_z + gh_z)
    n    = tanh(gi_n + r * gh_n)
    h    = n + z*(h - n);  pred = h @ w_out + b_out;  x_next = pred

Device-side restructurings:
- x_t = w_out.h_t + b_out exactly (t>=1), so the rank-1 input term for r/z
  folds INTO the recurrent weights: W'_rz = W_hh_rz^T + w_out u_rz^T (b_out
  absorbed into biases).  No per-step rank-1 matmuls.
- Gate matmuls run fp8-e4m3 DoubleRow (K=256/instr, 0.5 cyc/row).  Weights
  are split W_hi + W_lo (same scale) to cancel weight-quantization error; the
  moving operand is a shadow h8 = Q(8h) refreshed per step.  The bf16 state
  stays the elementwise source of truth (a pure-fp8 state diverges, 6.5e-2).
- Gate biases enter PSUM via K=1 ones-matmuls; the fp8 descale rides the
  ACT scale field and DVE per-partition scalars.
- x is produced broadcast over partitions by a matmul whose stationary is
  w_out replicated across output rows, streaming bf16 nn / zh (linearity:
  w_out.h' = w_out.nn + w_out.zh) so pred quality never touches fp8.
- preds for the OUTPUT are computed on the host from the DMA'd nn/zh
  tensors (f32 matvec per step); no PSUM row extraction on device.
- PSUM: gr/gz rotate in a [128,1024] f32 bufs=2 pool, gn [128,1024] bufs=1,
  xb [128,512] bufs=2 -> exactly 8 banks.
- Per-chunk engine budget: PE ~2985ns (16 MMs), ACT ~3114 (2 sigmoid+tanh),
  DVE ~2950 (rh stt, affine_then_add, h8 half), Pool ~2990 (h-update, h8
  half). nn/zh out-DMAs issue from the Pool queue (25ns) onto DMA engines.
"""

import sys

import numpy as np

if "/opt/trn_rl_repo" not in sys.path:
    sys.path.insert(0, "/opt/trn_rl_repo")

N = 16384
H = 256
T = 24
NCORES = 8
R = N // NCORES  # 2048 rows per core
CH = 512
NCH = R // CH

SW = 16.0            # fp8 weight scale (hi and lo at the same scale)
SH = 8.0             # fp8 h-shadow scale
DESC = 1.0 / (SW * SH)
IDESC = SW * SH

_CACHE: dict = {}


def _build():
    import concourse.bacc as bacc
    import concourse.mybir as mybir
    from concourse.tile import TileContext

    f32 = mybir.dt.float32
    bf16 = mybir.dt.bfloat16
    f8 = mybir.dt.float8e4
    AF = mybir.ActivationFunctionType
    OP = mybir.AluOpType
    DR = mybir.MatmulPerfMode.DoubleRow

    nc = bacc.Bacc()

    h0T_d = nc.declare_dram_parameter("h0T", [128, 2, R], bf16, isOutput=False)
    h08_d = nc.declare_dram_parameter("h08", [128, 2, R], f8, isOutput=False)
    wrz_d = nc.declare_dram_parameter("wrz", [128, 2, 2, 512], f8, isOutput=False)
    wrz0_d = nc.declare_dram_parameter("wrz0", [128, 2, 2, 512], f8, isOutput=False)
    wn_d = nc.declare_dram_parameter("wn", [128, 2, 2, 256], f8, isOutput=False)
    bias_d = nc.declare_dram_parameter("bias", [1, 8, 128], bf16, isOutput=False)
    worep_d = nc.declare_dram_parameter("worep", [128, 256], bf16, isOutput=False)
    # cols: 0,1 u_n/d | 2,3 c_nf/d | 4,5 c_n0/d | 6,7 b_n/d   (per half)
    scal_d = nc.declare_dram_parameter("scal", [128, 8], f32, isOutput=False)
    nn_d = nc.declare_dram_parameter("nn", [T, 128, 2, R], bf16, isOutput=True)
    zh_d = nc.declare_dram_parameter("zh", [T, 128, 2, R], bf16, isOutput=True)

    from contextlib import ExitStack

    with TileContext(nc) as tc, ExitStack() as stack:
        persist = stack.enter_context(tc.tile_pool(name="persist", bufs=1))

        def mk(shape, name, dt=bf16):
            return persist.tile(shape, dt, name=name, tag=name)

        hT = mk([128, 2, R], "hT")
        h8 = mk([128, 2, R], "h8", f8)
        nnP = [mk([128, 2, R], "nnA"), mk([128, 2, R], "nnB")]
        zhP = [mk([128, 2, R], "zhA"), mk([128, 2, R], "zhB")]
        wrz = mk([128, 2, 2, 512], "wrz", f8)
        wrz0 = mk([128, 2, 2, 512], "wrz0", f8)
        wn = mk([128, 2, 2, 256], "wn", f8)
        biasr = mk([1, 8, 128], "biasr")
        worep = mk([128, 256], "worep")
        scal = mk([128, 8], "scal", f32)
        ones = mk([1, CH], "ones")
        zeros = mk([128, CH], "zeros")

        # chunk-0 critical path first (h8, step-0 weights, biases), spread
        # across issue queues so the first matmul isn't ~6us out
        nc.vector.memset(ones[:], 1.0)
        nc.vector.memset(zeros[:], 0.0)
        nc.sync.dma_start(out=wrz0[:], in_=wrz0_d[:])
        nc.sync.dma_start(out=biasr[:], in_=bias_d[:])
        for cc in range(NCH):
            nc.sync.dma_start(out=h8[:, :, cc * CH : (cc + 1) * CH],
                              in_=h08_d[:, :, cc * CH : (cc + 1) * CH])
        nc.sync.dma_start(out=wn[:], in_=wn_d[:])
        nc.gpsimd.dma_start(out=scal[:], in_=scal_d[:])
        nc.gpsimd.dma_start(out=hT[:], in_=h0T_d[:])
        nc.gpsimd.dma_start(out=worep[:], in_=worep_d[:])
        nc.gpsimd.dma_start(out=wrz[:], in_=wrz_d[:])

        with (
            tc.tile_pool(name="grz", bufs=2, space="PSUM") as grzpool,
            tc.tile_pool(name="gn", bufs=1, space="PSUM") as gnpool,
            tc.tile_pool(name="xb", bufs=2, space="PSUM") as xbpool,
            tc.tile_pool(name="rz", bufs=3) as rzpool,
            tc.tile_pool(name="wk", bufs=8) as wkpool,
        ):
            pending = None

            def flush(p):
                """Gate chain tail for a chunk (runs one chunk late)."""
                t, c, gnt, xbt, rz2 = p
                nn = nnP[t % 2]
                zh = zhP[t % 2]
                sl = slice(c * CH, (c + 1) * CH)
                ta2 = wkpool.tile([128, 2, CH], bf16, tag="ta2", name="ta2")
                for hh in range(2):
                    rr = rz2[:, hh * CH : (hh + 1) * CH]
                    rh = wkpool.tile([128, CH], bf16, tag="rh", name="rh")
                    nc.vector.scalar_tensor_tensor(
                        rh[:], gnt[:, hh * CH : (hh + 1) * CH],
                        scal[:, 6 + hh : 7 + hh], rr, OP.add, OP.mult)
                    xin = xbt[:] if t > 0 else zeros[:]
                    bcol = (2 if t > 0 else 4) + hh
                    nc.vector.affine_then_add(
                        ta2[:, hh], xin, rh[:],
                        scal[:, hh : hh + 1], scal[:, bcol : bcol + 1])
                nc.scalar.activation(nn[:, :, sl], ta2[:], AF.Tanh, scale=DESC)
                last = t == T - 1
                for hh in range(2):
                    zz = rz2[:, (2 + hh) * CH : (3 + hh) * CH]
                    nslc = nn[:, hh, sl]
                    hslc = hT[:, hh, sl]
                    hmn = wkpool.tile([128, CH], bf16, tag="hmn", name="hmn")
                    nc.gpsimd.tensor_tensor(hmn[:], hslc, nslc, OP.subtract)
                    nc.gpsimd.tensor_tensor(zh[:, hh, sl], zz, hmn[:], OP.mult)
                    if not last:
                        # h_{T} itself is never consumed (host recovers preds
                        # from nn+zh) -- skip the state/shadow updates
                        nc.gpsimd.tensor_tensor(hslc, nslc, zh[:, hh, sl],
                                                OP.add)
                if not last:
                    nc.vector.tensor_scalar(h8[:, 0, sl], hT[:, 0, sl], SH, 0.0,
                                            OP.mult, OP.add)
                    nc.gpsimd.tensor_scalar(h8[:, 1, sl], hT[:, 1, sl], SH, 0.0,
                                            OP.mult, OP.add)
                if t == T - 1:
                    # tail: drain per chunk so the kernel doesn't end on one
                    # big serial DMA
                    nc.sync.dma_start(out=nn_d[t, :, :, sl], in_=nn[:, :, sl])
                    nc.sync.dma_start(out=zh_d[t, :, :, sl], in_=zh[:, :, sl])
                elif c == NCH - 1:
                    nc.sync.dma_start(out=nn_d[t], in_=nn[:])
                    nc.sync.dma_start(out=zh_d[t], in_=zh[:])

            for t in range(T):
                w = wrz0 if t == 0 else wrz
                bbase = 4 if t == 0 else 0
                nn = nnP[(t + 1) % 2]
                zh = zhP[(t + 1) % 2]
                for c in range(NCH):
                    sl = slice(c * CH, (c + 1) * CH)
                    xbt = None
                    if t > 0:
                        xbt = xbpool.tile([128, CH], f32, tag="xb", name="xb")
                        nc.tensor.matmul(xbt[:], worep[:, 0:128], nn[:, 0, sl],
                                         start=True, stop=False)
                        nc.tensor.matmul(xbt[:], worep[:, 128:256], nn[:, 1, sl],
                                         start=False, stop=False)
                        nc.tensor.matmul(xbt[:], worep[:, 0:128], zh[:, 0, sl],
                                         start=False, stop=False)
                        nc.tensor.matmul(xbt[:], worep[:, 128:256], zh[:, 1, sl],
                                         start=False, stop=True)
                    gr = grzpool.tile([128, 2 * CH], f32, tag="grz", name="gr")
                    gz = grzpool.tile([128, 2 * CH], f32, tag="grz", name="gz")
                    gnt = gnpool.tile([128, 2 * CH], f32, tag="gn", name="gn")
                    for j in range(4):
                        g = gr if j < 2 else gz
                        out = g[:, (j % 2) * CH : (j % 2 + 1) * CH]
                        nc.tensor.matmul(out, biasr[0:1, bbase + j, :], ones[:],
                                         start=True, stop=False)
                        nc.tensor.matmul(out, w[:, 0, :, j * 128 : (j + 1) * 128],
                                         h8[:, :, sl], start=False, stop=False,
                                         perf_mode=DR)
                        nc.tensor.matmul(out, w[:, 1, :, j * 128 : (j + 1) * 128],
                                         h8[:, :, sl], start=False, stop=True,
                                         perf_mode=DR)
                    for hh in range(2):
                        out = gnt[:, hh * CH : (hh + 1) * CH]
                        nc.tensor.matmul(out, wn[:, 0, :, hh * 128 : (hh + 1) * 128],
                                         h8[:, :, sl], start=True, stop=False,
                                         perf_mode=DR)
                        nc.tensor.matmul(out, wn[:, 1, :, hh * 128 : (hh + 1) * 128],
                                         h8[:, :, sl], start=False, stop=True,
                                         perf_mode=DR)
                    rz2 = rzpool.tile([128, 4 * CH], bf16, tag="rz2", name="rz2")
                    nc.scalar.activation(rz2[:, 0 : 2 * CH], gr[:], AF.Sigmoid,
                                         scale=DESC)
                    nc.scalar.activation(rz2[:, 2 * CH : 4 * CH], gz[:],
                                         AF.Sigmoid, scale=DESC)
                    if pending is not None:
                        flush(pending)
                    pending = (t, c, gnt, xbt, rz2)
            flush(pending)

    nc.compile()
    return nc


def _q8(x, scale):
    import ml_dtypes

    return (np.asarray(x, np.float32) * scale).astype(ml_dtypes.float8_e4m3)


def _prep_maps(encoder_out, w_proj, b_proj, W_ih, b_ih, W_hh, b_hh, w_out, b_out):
    import ml_dtypes

    f = np.float32
    bf = ml_dtypes.bfloat16
    W_hh = np.asarray(W_hh, f)
    w_out_f = np.asarray(w_out, f)
    u = (np.asarray(W_ih, f) @ np.asarray(w_proj, f)).astype(f)
    cvec = (np.asarray(W_ih, f) @ np.asarray(b_proj, f) + np.asarray(b_ih, f)).astype(f)
    b_hh = np.asarray(b_hh, f)
    bo = float(np.asarray(b_out, f)[0])

    WT = np.ascontiguousarray(W_hh.T)                # [256, 768]
    Wrz_fold = WT[:, : 2 * H] + np.outer(w_out_f, u[: 2 * H])
    Wrz_0 = WT[:, : 2 * H]
    Wn = WT[:, 2 * H :]
    brz_fold = b_hh[: 2 * H] + cvec[: 2 * H] + u[: 2 * H] * bo
    brz_0 = b_hh[: 2 * H] + cvec[: 2 * H]
    u_n = u[2 * H :]
    c_nf = cvec[2 * H :] + u_n * bo
    c_n0 = cvec[2 * H :]
    b_n = b_hh[2 * H :]

    def pack_w(W, m):
        hi = _q8(W, SW)
        lo = _q8(W - hi.astype(f) / SW, SW)
        out = np.zeros((128, 2, 2, m), ml_dtypes.float8_e4m3)
        for i, wq in enumerate((hi, lo)):
            out[:, i, 0, :] = wq[0:128, :]
            out[:, i, 1, :] = wq[128:256, :]
        return out

    wrz = pack_w(Wrz_fold, 512)
    wrz0 = pack_w(Wrz_0, 512)
    wn = pack_w(Wn, 256)

    bias = np.zeros((1, 8, 128), f)
    for j in range(4):
        bias[0, j] = brz_fold[j * 128 : (j + 1) * 128] * IDESC
        bias[0, 4 + j] = brz_0[j * 128 : (j + 1) * 128] * IDESC
    bias = bias.astype(bf)

    worep = np.zeros((128, 256), f)
    worep[:, 0:128] = np.tile(w_out_f[0:128][:, None], (1, 128))
    worep[:, 128:256] = np.tile(w_out_f[128:256][:, None], (1, 128))
    worep = worep.astype(bf)

    scal = np.zeros((128, 8), f)
    scal[:, 0] = u_n[0:128] * IDESC
    scal[:, 1] = u_n[128:256] * IDESC
    scal[:, 2] = c_nf[0:128] * IDESC
    scal[:, 3] = c_nf[128:256] * IDESC
    scal[:, 4] = c_n0[0:128] * IDESC
    scal[:, 5] = c_n0[128:256] * IDESC
    scal[:, 6] = b_n[0:128] * IDESC
    scal[:, 7] = b_n[128:256] * IDESC

    enc = np.asarray(encoder_out, f)
    maps = []
    for i in range(NCORES):
        blk = enc[i * R : (i + 1) * R].T  # [256, R]
        h0T = np.zeros((128, 2, R), f)
        h0T[:, 0, :] = blk[0:128]
        h0T[:, 1, :] = blk[128:256]
        maps.append(dict(
            h0T=h0T.astype(bf),
            h08=(h0T * SH).astype(ml_dtypes.float8_e4m3),
            wrz=wrz, wrz0=wrz0, wn=wn, bias=bias, worep=worep, scal=scal,
        ))
    return maps, (w_out_f.astype(bf).astype(f), bo)


def _run(inputs, trace=False, **kw):
    import time

    from concourse.bass_utils import run_bass_kernel_spmd

    if "nc" not in _CACHE:
        _CACHE["nc"] = _build()
    nc = _CACHE["nc"]
    in_maps, (wo_h, bo) = _prep_maps(**inputs)
    res = None
    for attempt, pause in enumerate((0, 30, 120)):
        if pause:
            time.sleep(pause)  # transient NRT/axon device errors self-recover
        try:
            res = run_bass_kernel_spmd(nc, in_maps, core_ids=list(range(NCORES)),
                                       trace=trace, **kw)
            break
        except Exception:
            if attempt == 2:
                raise
    full = np.empty((N, T), np.float32)
    for i in range(NCORES):
        nn_o = np.asarray(res.results[i]["nn"]).astype(np.float32)  # [T,128,2,R]
        zh_o = np.asarray(res.results[i]["zh"]).astype(np.float32)
        hsum = nn_o + zh_o                                           # h_{t+1}
        h_flat = hsum.transpose(0, 3, 2, 1).reshape(T, R, 2 * 128)
        preds = h_flat @ wo_h + bo                                   # [T, R]
        full[i * R : (i + 1) * R] = preds.T
    return full, res


def kernel(**inputs):
    inputs = {k: np.asarray(v) for k, v in inputs.items()}
    full, _ = _run(inputs)
    return full
